# revision 3
# baseline (speedup 1.0000x reference)
"""GNN message-passing kernel for Trainium2 (8 NeuronCores, batch-parallel).

Computation (per reference):
    norm_adj = adjacency * dinv * dinv.T + I            [10,10]   (host, O(100) flops)
    support  = einsum('bcf,fo->bco', x, kernel)         [B,C,O]
    out      = elu(einsum('ij,bjo->bio', norm_adj, support) + bias)
    out      = (out - mean) * rsqrt(var+eps) * gamma + beta

Device strategy per core (512 batches = 5120 rows of [b,c] x f), all bf16
matmul operands:
  1. "Transposing mix": PE matmul with x-chunks [crows<=120, 128f] as the
     stationary operand and a block-diagonal norm_adj matrix [crows, crows]
     as the moving operand. One op both applies the channel mix and lands
     the activations transposed ([f, rows]) as needed by the main matmul.
  2. Main matmul: outT[o,rows] += K[f,o].T @ yT[f,rows], kernel matrix
     resident in SBUF as bf16 (loaded once, outside the repeat loop).
  3. Epilogue (bf16 intermediates for 2-4x DVE tiers):
     e = scale*exp(zb) on ACT; t0 = relu(zb)+1, s = min(scale*t0, e),
     fin = s + (shift-scale) on DVE; outT DMA from the SP queue.

Pipelining: the mix work (x-chunk DMAs + mix matmuls + PSUM->SBUF copies)
for global panel g+1 is interleaved into the 16 ot-slots of panel g's main
matmul, across repeat boundaries, so the PE never sits idle waiting for a
standalone mix phase. Only the very first panel's mix runs as a prologue.
"""

from contextlib import ExitStack

import numpy as np
import ml_dtypes

import concourse.bass as bass
import concourse.bacc as bacc
import concourse.mybir as mybir
import concourse.tile as tile
from concourse.bass_utils import run_bass_kernel_spmd

F32 = mybir.dt.float32
BF16 = mybir.dt.bfloat16
ALU = mybir.AluOpType
ACTF = mybir.ActivationFunctionType
NPBF16 = ml_dtypes.bfloat16

P = 128
BN_EPS = 1e-3
N_CORES = 8
C = 10  # channels
BDW = 128  # mix moving-operand pad width (block-diag storage stride)
CHUNKS = (12,) * 10 + (8,)   # batches per mix chunk; 128 batches = 1280 rows
JT = (512, 512, 256)         # main matmul j-tile widths


def build_nc(rows, F, O, repeats=1, n_cores=N_CORES):
    panel = sum(CHUNKS) * C      # 1280
    assert rows % panel == 0
    n_panels = rows // panel
    FT, OT = F // P, O // P
    bd_sizes = sorted({nb * C for nb in CHUNKS})
    G = repeats * n_panels       # global panel count

    nc = bacc.Bacc("TRN2", target_bir_lowering=False, debug=False,
                   enable_asserts=False, num_devices=n_cores)
    x_d = nc.dram_tensor("x_local", [rows, F], BF16, kind="ExternalInput").ap()
    k_d = nc.dram_tensor("kern", [F, O], BF16, kind="ExternalInput").ap()
    bdb_d = nc.dram_tensor("bdb", [P, BDW * len(bd_sizes)], BF16,
                           kind="ExternalInput").ap()
    prm_d = nc.dram_tensor("prm", [P, 4 * OT], F32, kind="ExternalInput").ap()
    outT_d = nc.dram_tensor("outT", [O, rows], BF16, kind="ExternalOutput").ap()

    with tile.TileContext(nc) as tc, ExitStack() as ctx:
        const = ctx.enter_context(tc.tile_pool(name="const", bufs=1))
        bdb = const.tile([P, BDW * len(bd_sizes)], BF16, name="bdb")
        prm = const.tile([P, 4 * OT], F32, name="prm")
        nc.sync.dma_start(bdb, bdb_d)
        nc.sync.dma_start(prm, prm_d)
        bd_t = {sz: bdb[:sz, BDW * i : BDW * i + sz]
                for i, sz in enumerate(bd_sizes)}
        kb = [const.tile([P, O], BF16, name=f"kb{fb}", tag=f"kb{fb}")
              for fb in range(FT)]
        for fb in range(FT):
            nc.scalar.dma_start(kb[fb], k_d[fb * P : (fb + 1) * P, :])

        xpool = ctx.enter_context(tc.tile_pool(name="xpool", bufs=6))
        ypool = ctx.enter_context(tc.tile_pool(name="ypool", bufs=2))
        mixps = ctx.enter_context(tc.tile_pool(name="mixps", bufs=2, space="PSUM"))
        mainps = ctx.enter_context(tc.tile_pool(name="mainps", bufs=6, space="PSUM"))
        tmp = ctx.enter_context(tc.tile_pool(name="tmp", bufs=3))
        fpool = ctx.enter_context(tc.tile_pool(name="fpool", bufs=4))

        ytall = {}   # global panel -> SBUF tile [P, FT, panel]
        xts = {}     # (g, ci) -> x chunk tile

        def emit_mix_dma(g, ci):
            pi = g % n_panels
            coff = sum(cb * C for cb in CHUNKS[:ci])
            crows = CHUNKS[ci] * C
            row0 = pi * panel + coff
            xt = xpool.tile([120, F], BF16, name=f"x_{g}_{ci}", tag="xc")[:crows]
            xts[(g, ci)] = xt
            nc.sync.dma_start(xt, x_d[row0 : row0 + crows, :])

        def emit_mix_unit(g, ci, fbp):
            crows = CHUNKS[ci] * C
            coff = sum(cb * C for cb in CHUNKS[:ci])
            xt = xts[(g, ci)]
            fb = 4 * fbp
            ps = mixps.tile([P, 4, 120], F32, name=f"mps_{g}_{ci}_{fbp}",
                            tag="mixps")[:, :, :crows]
            for q in range(4):
                nc.tensor.matmul(
                    ps[:, q, :],
                    lhsT=xt[:, (fb + q) * P : (fb + q + 1) * P],
                    rhs=bd_t[crows],
                    start=True, stop=True,
                )
            dst = ytall[g][:, fb : fb + 4, coff : coff + crows]
            # steady state: ACT (DVE carries the epilogue); prologue: alternate
            # DVE/ACT so the standalone mix drains at 2-engine rate.
            if g == 0 and (ci * 4 + fbp) % 2 == 0:
                nc.vector.tensor_copy(dst, ps)
            else:
                nc.scalar.activation(dst, ps, ACTF.Copy)

        def mix_items(g):
            ytall[g] = ypool.tile([P, FT, panel], BF16, name=f"yt_{g}", tag="yt")
            items = [lambda g=g: emit_mix_dma(g, 0), lambda g=g: emit_mix_dma(g, 1)]
            for ci in range(len(CHUNKS)):
                if ci + 2 < len(CHUNKS):
                    items.append(lambda g=g, c=ci + 2: emit_mix_dma(g, c))
                for fbp in range(FT // 4):
                    items.append(lambda g=g, c=ci, f=fbp: emit_mix_unit(g, c, f))
            return items

        # ---- prologue: first panel's mix runs standalone
        for it in mix_items(0):
            it()

        for g in range(G):
            pi = g % n_panels
            row0 = pi * panel
            yt = ytall[g]
            items = mix_items(g + 1) if g + 1 < G else []
            emitted = 0
            for ot in range(OT):
                pss = []
                joffs = []
                joff = 0
                for ji, jw in enumerate(JT):
                    pss.append(mainps.tile([P, 512], F32, name=f"ops_{g}_{ot}_{ji}",
                                           tag="mainps")[:, :jw])
                    joffs.append(joff)
                    joff += jw
                for fb in range(FT):
                    for ji, jw in enumerate(JT):
                        nc.tensor.matmul(
                            pss[ji],
                            lhsT=kb[fb][:, ot * P : (ot + 1) * P],
                            rhs=yt[:, fb, joffs[ji] : joffs[ji] + jw],
                            start=(fb == 0), stop=(fb == FT - 1),
                        )
                bias2_ap = prm[:, ot : ot + 1]
                bias1_ap = prm[:, OT + ot : OT + ot + 1]
                scale_ap = prm[:, 2 * OT + ot : 2 * OT + ot + 1]
                shift_ap = prm[:, 3 * OT + ot : 3 * OT + ot + 1]
                for ji, jw in enumerate(JT):
                    ps = pss[ji]
                    joff = joffs[ji]
                    e = tmp.tile([P, 512], BF16, name=f"e_{g}_{ot}_{ji}", tag="e")[:, :jw]
                    t0 = tmp.tile([P, 512], BF16, name=f"t0_{g}_{ot}_{ji}", tag="t0")[:, :jw]
                    s = tmp.tile([P, 512], BF16, name=f"s_{g}_{ot}_{ji}", tag="s")[:, :jw]
                    fin = fpool.tile([P, 512], BF16, name=f"fin_{g}_{ot}_{ji}", tag="fin")[:, :jw]
                    nc.scalar.activation(e, ps, ACTF.Exp, bias=bias2_ap)
                    nc.vector.tensor_scalar(t0, ps, bias1_ap, 1.0, op0=ALU.add, op1=ALU.max)
                    nc.vector.scalar_tensor_tensor(s, in0=t0, scalar=scale_ap, in1=e,
                                                   op0=ALU.mult, op1=ALU.min)
                    nc.vector.tensor_scalar(fin, s, shift_ap, None, op0=ALU.add)
                    nc.sync.dma_start(
                        outT_d[ot * P : (ot + 1) * P, row0 + joff : row0 + joff + jw], fin)
                # interleave next panel's mix work into this ot-slot
                want = (len(items) * (ot + 1) + OT - 1) // OT
                while emitted < min(want, len(items)):
                    items[emitted]()
                    emitted += 1
    nc.compile()
    return nc


def _host_prep(adjacency, bias, gamma, beta, moving_mean, moving_var,
               chunk_batches=(12, 12, 8), O=2048):
    """Build the tiny derived inputs on the host: (bdb bf16, prm fp32)."""
    A = np.asarray(adjacency, np.float32)
    deg = np.maximum(np.abs(A).sum(axis=1, keepdims=True), 1e-8)
    dinv = deg ** -0.5
    na = A * dinv * dinv.T + np.eye(C, dtype=np.float32)  # [10,10]

    bd_sizes = sorted({nb * C for nb in chunk_batches})
    OT = O // P
    bdb = np.zeros((P, BDW * len(bd_sizes)), np.float32)
    for i, sz in enumerate(bd_sizes):
        nb = sz // C
        for g in range(nb):
            bdb[g * C : (g + 1) * C, BDW * i + g * C : BDW * i + (g + 1) * C] = na.T
    prm = np.zeros((P, 4 * OT), np.float32)
    scale = np.asarray(gamma, np.float32) / np.sqrt(np.asarray(moving_var, np.float32) + BN_EPS)
    assert (scale > 0).all(), "epilogue requires positive BN scale"
    shift2 = np.asarray(beta, np.float32) - np.asarray(moving_mean, np.float32) * scale - scale
    b = np.asarray(bias, np.float32)
    prm[:, 0:OT] = (b + np.log(scale)).reshape(OT, P).T
    prm[:, OT : 2 * OT] = (b + 1.0).reshape(OT, P).T
    prm[:, 2 * OT : 3 * OT] = scale.reshape(OT, P).T
    prm[:, 3 * OT : 4 * OT] = shift2.reshape(OT, P).T
    return bdb.astype(NPBF16), prm


def make_in_maps(x, adjacency, kernel, bias, gamma, beta, moving_mean,
                 moving_var, chunk_batches=(12, 12, 8)):
    B, C_, F = x.shape
    O = kernel.shape[1]
    assert C_ == C and B % N_CORES == 0
    bl = B // N_CORES
    rows = bl * C
    bdb, prm = _host_prep(adjacency, bias, gamma, beta, moving_mean,
                          moving_var, chunk_batches, O)
    kern_np = np.ascontiguousarray(np.asarray(kernel, np.float32)).astype(NPBF16)
    x_np = np.asarray(x, np.float32).astype(NPBF16)
    in_maps = []
    for c in range(N_CORES):
        in_maps.append({
            "x_local": np.ascontiguousarray(x_np[c * bl : (c + 1) * bl].reshape(rows, F)),
            "kern": kern_np,
            "bdb": bdb,
            "prm": prm,
        })
    return in_maps


def kernel(x, adjacency, kernel, bias, gamma, beta, moving_mean, moving_var):
    B, C_, F = x.shape
    O = kernel.shape[1]
    bl = B // N_CORES
    rows = bl * C

    chunk_batches = (12, 12, 8)
    in_maps = make_in_maps(x, adjacency, kernel, bias, gamma, beta,
                           moving_mean, moving_var, chunk_batches)
    nc = build_nc(rows, F, O)
    res = run_bass_kernel_spmd(nc, in_maps, core_ids=list(range(N_CORES)), trace=False)

    out = np.empty((B, C, O), np.float32)
    for c in range(N_CORES):
        outT = np.asarray(res.results[c]["outT"], dtype=np.float32)  # [O, rows]
        out[c * bl : (c + 1) * bl] = outT.T.reshape(bl, C, O)
    return out
